# revision 1
# baseline (speedup 1.0000x reference)
"""Trainium2 Bass kernel for a tanh-RNN (Elman) with output projection.

Reference semantics (fp32):
    W_x = W_ih[:, :1024]; W_h = W_ih[:, 1024:]
    h_t   = tanh(x_t @ W_x.T + h_{t-1} @ W_h.T + b_ih)     # (B, H)
    out_t = h_t @ W_ho.T + b_ho                            # (B, O)
Shapes: x (512, 64, 1024), h0 (64, 1024), W_ih (1024, 2048), b_ih (1024,),
W_ho (512, 1024), b_ho (512,) -> out (512, 64, 512).

Strategy: data-parallel over batch (64 -> 8 per core on 8 NeuronCores),
weights replicated. Per core, everything is computed in a "transposed"
layout with the contraction dim on SBUF partitions:
  phase 1: U^T = W_x @ x^T + b_ih for all timesteps (big matmuls, bf16)
  phase 2: 512 serial steps, h^T kept as one [128, 64] column block
           (col = ho*8 + b): z = sum_hi WhT.T @ h via 64 weight-stationary
           matmuls/step into one [128, 64] PSUM tile (8 accumulation
           groups), then one DVE add of U and one ScalarE tanh.
  phase 3: out^T = W_ho @ h^T over all timesteps, then PE-transpose to
           row-major and DMA out.
bf16 PE operands / fp32 PSUM accumulation (end-to-end rel err ~3e-3).
"""
import sys
sys.path.insert(0, "/opt/trn_rl_repo")

import numpy as np

from concourse import bacc
import concourse.mybir as mybir
from concourse.tile import TileContext
from concourse.masks import make_identity

N_CORES = 8
S = 512
B = 64
B_LOC = B // N_CORES          # 8
I = 1024
H = 1024
O = 512
KI = I // 128                 # 8 i-tiles
KH = H // 128                 # 8 h-tiles
KO = O // 128                 # 4 o-tiles
R = S * B_LOC                 # 4096 rows (t-major, b-minor)
RC = 512                      # rows per phase-1/3 chunk
TB = 16                       # phase-2 steps per block
HB = KH * B_LOC               # 64 columns of the h^T block (ho*8 + b)
BF = mybir.dt.bfloat16
F32 = mybir.dt.float32


def _load_wT(nc, pool_nat, pool_w, src_rows, col_off, n_row_tiles, n_col_tiles, name):
    """Load weight [rows, cols] from DRAM (fp32) -> transposed bf16 tiles.

    Returns wT[(k, m)] tiles of [128 contraction-part, 128 out-free]:
    wT[(k, m)][i, j] = W[m*128 + j, col_off + k*128 + i].
    """
    wT = {}
    for m in range(n_row_tiles):
        nat = pool_nat.tile([128, n_col_tiles * 128], BF, tag="wnat", name="wnat")
        nc.gpsimd.dma_start(
            out=nat[:],
            in_=src_rows[m * 128:(m + 1) * 128,
                         col_off:col_off + n_col_tiles * 128],
        )
        for k in range(n_col_tiles):
            t = pool_w.tile([128, 128], BF, tag=f"{name}_{k}_{m}",
                            name=f"{name}_{k}_{m}")
            nc.sync.dma_start(
                out=t[:], in_=nat[:, k * 128:(k + 1) * 128], transpose=True
            )
            wT[(k, m)] = t
    return wT


def build_nc(s_run=S, p2_reps=1, timing_no_act=False, fused_tail=False,
             tb=TB, pz_bufs=6, big_bufs=1, tr_bufs=1, p2_bufs=2):
    nc = bacc.Bacc(None, target_bir_lowering=False, debug=False)
    x = nc.dram_tensor("x", [R, I], F32, kind="ExternalInput")
    h0 = nc.dram_tensor("h0", [B_LOC, H], F32, kind="ExternalInput")
    w_ih = nc.dram_tensor("w_ih", [H, I + H], F32, kind="ExternalInput")
    b_ih = nc.dram_tensor("b_ih", [H], F32, kind="ExternalInput")
    w_ho = nc.dram_tensor("w_ho", [O, H], F32, kind="ExternalInput")
    b_ho = nc.dram_tensor("b_ho", [O], F32, kind="ExternalInput")
    out = nc.dram_tensor("out", [R, O], F32, kind="ExternalOutput")

    # U^T in "flat h block" layout: [p][t][ho][b] so a step's U is [128, 64]
    u_buf = nc.dram_tensor("u_buf", [128, S, KH, B_LOC], F32)
    h_buf = nc.dram_tensor("h_buf", [KH, 128, R], BF)     # h^T history

    n_chunks = (s_run * B_LOC) // RC
    n_blocks = s_run // tb
    tpc = RC // B_LOC            # timesteps per phase-1/3 chunk (64)

    with TileContext(nc) as tc:
        with (
            tc.tile_pool(name="wnat", bufs=2) as pool_nat,
            tc.tile_pool(name="weights", bufs=1) as pool_w,
            tc.tile_pool(name="small", bufs=1) as pool_small,
            tc.tile_pool(name="p1x", bufs=2) as pool_x,
            tc.tile_pool(name="p1u", bufs=3) as pool_u,
            tc.tile_pool(name="p2", bufs=p2_bufs) as pool_p2,
            tc.tile_pool(name="p3", bufs=2) as pool_p3,
            tc.tile_pool(name="ps_a", bufs=pz_bufs, space="PSUM") as ps_a,
            tc.tile_pool(name="ps_big", bufs=big_bufs, space="PSUM") as ps_big,
            tc.tile_pool(name="ps_tr", bufs=tr_bufs, space="PSUM") as ps_tr,
        ):
            # ---- weight preload (transposed bf16 tiles) ----
            wxT = _load_wT(nc, pool_nat, pool_w, w_ih, 0, KH, KI, "wx")
            whT = _load_wT(nc, pool_nat, pool_w, w_ih, I, KH, KH, "wh")
            woT = _load_wT(nc, pool_nat, pool_w, w_ho, 0, KO, KH, "wo")

            bih_t = {}
            for m in range(KH):
                t = pool_small.tile([128, 1], F32, tag=f"bih{m}", name=f"bih{m}")
                nc.sync.dma_start(out=t[:], in_=b_ih[m * 128:(m + 1) * 128][:, None])
                bih_t[m] = t
            bho_t = {}
            for m in range(KO):
                t = pool_small.tile([128, 1], F32, tag=f"bho{m}", name=f"bho{m}")
                nc.sync.dma_start(out=t[:], in_=b_ho[m * 128:(m + 1) * 128][:, None])
                bho_t[m] = t

            ident = pool_small.tile([128, 128], F32, tag="ident")
            make_identity(nc, ident[:])

            # h0 block [128, 64] bf16: col ho*8+b holds h0[b, ho*128+p]
            h0blk = pool_small.tile([128, HB], BF, tag="h0blk")
            for k in range(KH):
                nc.gpsimd.dma_start(
                    out=h0blk[:, k * B_LOC:(k + 1) * B_LOC],
                    in_=h0[:, k * 128:(k + 1) * 128].rearrange("b h -> h b"),
                )

            # ---- phase 1: U^T = W_x @ x^T + b_ih ----
            for c in range(n_chunks):
                xT = {}
                for k in range(KI):
                    xT[k] = pool_x.tile([128, RC], BF, tag=f"xT{k}", name=f"xT{k}")
                for rb in range(RC // 128):
                    xrow = pool_x.tile([128, I], BF, tag="xrow")
                    r0 = c * RC + rb * 128
                    nc.gpsimd.dma_start(out=xrow[:], in_=x[r0:r0 + 128, :])
                    for k in range(KI):
                        nc.sync.dma_start(
                            out=xT[k][:, rb * 128:(rb + 1) * 128],
                            in_=xrow[:, k * 128:(k + 1) * 128],
                            transpose=True,
                        )
                for ho in range(KH):
                    pz = ps_big.tile([128, RC], F32, tag="pbig", name="p1z")
                    for k in range(KI):
                        nc.tensor.matmul(
                            pz[:], wxT[(k, ho)][:], xT[k][:],
                            start=(k == 0), stop=(k == KI - 1),
                        )
                    usb = pool_u.tile([128, RC], F32, tag="usb")
                    nc.scalar.activation(
                        usb[:], pz[:], mybir.ActivationFunctionType.Identity,
                        bias=bih_t[ho][:],
                    )
                    nc.sync.dma_start(
                        out=u_buf[:, c * tpc:(c + 1) * tpc, ho, :],
                        in_=usb[:],
                    )

            # ---- phase 2: serial recurrence on the [128, 64] h block ----
            for _rep in range(p2_reps):
              prev_blk = h0blk
              prev_off = 0
              for tb_ in range(n_blocks):
                  usb2 = pool_p2.tile([128, tb * HB], F32, tag="u2", name="u2")
                  nc.sync.dma_start(
                      out=usb2[:],
                      in_=u_buf[:, tb_ * tb:(tb_ + 1) * tb, :, :],
                  )
                  histf = pool_p2.tile([128, tb * HB], BF, tag="histf", name="histf")
                  for s_ in range(tb):
                      off = s_ * HB
                      if fused_tail:
                          pzf = ps_a.tile([128, HB], F32, tag="pz", name="pz")
                          for ho in range(KH):
                              pslf = pzf[:, ho * B_LOC:(ho + 1) * B_LOC]
                              for hi in range(KH):
                                  rhs = prev_blk[:, prev_off + hi * B_LOC:
                                                 prev_off + (hi + 1) * B_LOC]
                                  nc.tensor.matmul(
                                      pslf, whT[(hi, ho)][:], rhs,
                                      start=(hi == 0), stop=(hi == KH - 1),
                                  )
                          nc.vector.tensor_add(pzf[:], pzf[:], usb2[:, off:off + HB])
                          nc.scalar.activation(
                              histf[:, off:off + HB], pzf[:],
                              mybir.ActivationFunctionType.Tanh,
                          )
                          prev_blk = histf
                          prev_off = off
                          continue
                      # per-ho accumulation groups; each group's U-add and
                      # tanh pipeline behind the next group's matmuls, so
                      # only the last group's tail is on the critical path
                      for ho in range(KH):
                          pz = ps_a.tile([128, B_LOC], F32, tag="pz", name="pz")
                          for hi in range(KH):
                              if timing_no_act:
                                  rhs = h0blk[:, hi * B_LOC:(hi + 1) * B_LOC]
                              else:
                                  rhs = prev_blk[:, prev_off + hi * B_LOC:
                                                 prev_off + (hi + 1) * B_LOC]
                              nc.tensor.matmul(
                                  pz[:], whT[(hi, ho)][:], rhs,
                                  start=(hi == 0), stop=(hi == KH - 1),
                              )
                          osl = slice(off + ho * B_LOC, off + (ho + 1) * B_LOC)
                          nc.vector.tensor_add(
                              pz[:], pz[:], usb2[:, osl]
                          )
                          if not timing_no_act:
                              nc.scalar.activation(
                                  histf[:, osl], pz[:],
                                  mybir.ActivationFunctionType.Tanh,
                              )
                      prev_blk = histf
                      prev_off = off
                  if not timing_no_act:
                      # spill h^T history for phase 3 (per k-tile layout)
                      hview = histf[:].rearrange("p (s k b) -> p s (k b)", k=KH, b=B_LOC)
                      for k in range(KH):
                          nc.sync.dma_start(
                              out=h_buf[k][:, tb_ * tb * B_LOC:(tb_ + 1) * tb * B_LOC],
                              in_=hview[:, :, k * B_LOC:(k + 1) * B_LOC],
                          )

            # ---- phase 3: out^T = W_ho @ h^T, transpose, store ----
            for c in range(n_chunks):
                hsb = {}
                for k in range(KH):
                    hsb[k] = pool_p3.tile([128, RC], BF, tag=f"hsb{k}", name=f"hsb{k}")
                    nc.sync.dma_start(
                        out=hsb[k][:], in_=h_buf[k][:, c * RC:(c + 1) * RC]
                    )
                oT = {}
                for o in range(KO):
                    po = ps_big.tile([128, RC], F32, tag="pbig", name="p3z")
                    for k in range(KH):
                        nc.tensor.matmul(
                            po[:], woT[(k, o)][:], hsb[k][:],
                            start=(k == 0), stop=(k == KH - 1),
                        )
                    oT[o] = pool_p3.tile([128, RC], F32, tag=f"oT{o}", name=f"oT{o}")
                    nc.scalar.activation(
                        oT[o][:], po[:], mybir.ActivationFunctionType.Identity,
                        bias=bho_t[o][:],
                    )
                for rb in range(RC // 128):
                    osb = pool_p3.tile([128, O], F32, tag="osb")
                    for o in range(KO):
                        ptr = ps_tr.tile([128, 128], F32, tag="ptr")
                        nc.tensor.transpose(
                            ptr[:], oT[o][:, rb * 128:(rb + 1) * 128], ident[:]
                        )
                        nc.vector.tensor_copy(
                            osb[:, o * 128:(o + 1) * 128], ptr[:]
                        )
                    r0 = c * RC + rb * 128
                    nc.sync.dma_start(out=out[r0:r0 + 128, :], in_=osb[:])

    nc.compile()
    return nc


_NC_CACHE = {}


def _get_nc():
    if "nc" not in _NC_CACHE:
        _NC_CACHE["nc"] = build_nc()
    return _NC_CACHE["nc"]


def make_in_maps(x, h0, W_ih, b_ih, W_ho, b_ho):
    in_maps = []
    for c in range(N_CORES):
        bsl = slice(c * B_LOC, (c + 1) * B_LOC)
        in_maps.append({
            "x": np.ascontiguousarray(x[:, bsl, :].reshape(R, I)),
            "h0": np.ascontiguousarray(h0[bsl]),
            "w_ih": W_ih,
            "b_ih": b_ih,
            "w_ho": W_ho,
            "b_ho": b_ho,
        })
    return in_maps


def kernel(x, h0, W_ih, b_ih, W_ho, b_ho):
    x = np.ascontiguousarray(np.asarray(x, dtype=np.float32))
    h0 = np.ascontiguousarray(np.asarray(h0, dtype=np.float32))
    W_ih = np.ascontiguousarray(np.asarray(W_ih, dtype=np.float32))
    b_ih = np.ascontiguousarray(np.asarray(b_ih, dtype=np.float32))
    W_ho = np.ascontiguousarray(np.asarray(W_ho, dtype=np.float32))
    b_ho = np.ascontiguousarray(np.asarray(b_ho, dtype=np.float32))

    from concourse.bass_utils import run_bass_kernel_spmd

    nc = _get_nc()
    in_maps = make_in_maps(x, h0, W_ih, b_ih, W_ho, b_ho)
    res = run_bass_kernel_spmd(nc, in_maps, list(range(N_CORES)))
    out = np.empty((S, B, O), np.float32)
    for c in range(N_CORES):
        out[:, c * B_LOC:(c + 1) * B_LOC, :] = (
            res.results[c]["out"].reshape(S, B_LOC, O)
        )
    return out

